# revision 12
# baseline (speedup 1.0000x reference)
"""CountMamba single-step kernel for 8 Trainium2 NeuronCores.

Sharding: tensor-parallel over Mamba heads (NH=8 == n_cores, head h -> core h)
and CNN output channels (96/core). Residual stream replicated on all cores.
Per Mamba layer: one AllReduce of [769, 32] (out_proj partials + RMS sum-sq).
CNN: AllGather after each of the 4 conv layers.

All weights are pre-transposed on the host so every device DMA is a natural
(contiguous) load; the PE consumes them as stationary lhsT tiles.
Activations are channel-major [ch(part), batch(free)] throughout.
"""
import sys
sys.path.insert(0, '/opt/trn_rl_repo')
import numpy as np

B = 32; P_IN = 16; W_IN = 6; D = 768; DEPTH = 12; K = 5
DI = 1536; DCONV = 1792; NH = 8; HD = 192; DS = 128; NCLS = 100
NCORES = 8
OC = D // NCORES          # 96 cnn out-channels per core
NCH = 6                   # 768 / 128 chunks

F32 = None  # set after import

_built = {}


def _build():
    """Build + compile the Bass program once per process."""
    if 'nc' in _built:
        return _built['nc']
    import concourse.bacc as bacc
    import concourse.mybir as mybir
    import concourse.tile as tile

    f32 = mybir.dt.float32
    AF = mybir.ActivationFunctionType
    ALU = mybir.AluOpType

    nc = bacc.Bacc("TRN2", target_bir_lowering=False, debug=False,
                   num_devices=NCORES)

    # ---------------- DRAM inputs (per-core, host-prepped) ----------------
    t_xT = nc.dram_tensor("xT", [P_IN, W_IN * B], f32, kind="ExternalInput")
    t_pwT = nc.dram_tensor("pwT", [P_IN, D], f32, kind="ExternalInput")
    t_pb = nc.dram_tensor("pb", [128, NCH], f32, kind="ExternalInput")
    t_cwT = nc.dram_tensor("cwT", [4, 128, K, NCH, OC], f32, kind="ExternalInput")
    t_caff = nc.dram_tensor("caff", [OC, 4, 2], f32, kind="ExternalInput")
    t_cst = nc.dram_tensor("cst", [4, NCH, 128, 4, B], f32, kind="ExternalInput")
    t_pos = nc.dram_tensor("pos", [128, NCH], f32, kind="ExternalInput")
    t_w1T = nc.dram_tensor("w1T", [DEPTH, NCH, 128, 768], f32, kind="ExternalInput")
    t_cp = nc.dram_tensor("cp", [DEPTH, 128, 4, K], f32, kind="ExternalInput")
    t_mcs = nc.dram_tensor("mcs", [DEPTH, 4, 128, 3, B], f32, kind="ExternalInput")
    t_ssmT = nc.dram_tensor("ssmT", [DEPTH, B, DS, HD], f32, kind="ExternalInput")
    t_w2T = nc.dram_tensor("w2T", [DEPTH, HD, D], f32, kind="ExternalInput")
    t_lp = nc.dram_tensor("lp", [1, DEPTH * 4], f32, kind="ExternalInput")
    t_fcn = nc.dram_tensor("fcn", [128, NCH, 2], f32, kind="ExternalInput")
    t_hist = nc.dram_tensor("histT", [128, NCH, B], f32, kind="ExternalInput")
    t_fcwT = nc.dram_tensor("fcwT", [128, NCH, NCLS], f32, kind="ExternalInput")
    t_fcb = nc.dram_tensor("fcb", [1, NCLS], f32, kind="ExternalInput")
    t_iden = nc.dram_tensor("iden", [128, 128], f32, kind="ExternalInput")

    t_logits = nc.dram_tensor("logits", [B, NCLS], f32, kind="ExternalOutput")
    t_histo = nc.dram_tensor("history", [B, D], f32, kind="ExternalOutput")

    RG = [list(range(NCORES))]

    with tile.TileContext(nc) as tc:
        with tc.tile_pool(name="const", bufs=1) as cpool, \
             tc.tile_pool(name="cnn", bufs=2) as npool, \
             tc.tile_pool(name="wts", bufs=2) as wpool, \
             tc.tile_pool(name="act", bufs=1) as apool, \
             tc.tile_pool(name="ps", bufs=1, space="PSUM") as pspool, \
             tc.tile_pool(name="ps2", bufs=1, space="PSUM") as ps2pool, \
             tc.tile_pool(name="dram", bufs=2, space="DRAM") as dpool:

            # ---- constants ----
            ones_c = cpool.tile([128, 1], f32)     # column of ones
            nc.vector.memset(ones_c[:], 1.0)
            ones_r = cpool.tile([1, 128], f32)     # row of ones (partition 0)
            nc.vector.memset(ones_r[:], 1.0)
            zero_c = cpool.tile([128, 1], f32)
            nc.vector.memset(zero_c[:], 0.0)
            eps_c = cpool.tile([128, 1], f32)
            nc.vector.memset(eps_c[:], 1e-5)
            ones_f = cpool.tile([128, 128], f32)
            nc.vector.memset(ones_f[:], 1.0)
            iden = cpool.tile([128, 128], f32)
            nc.sync.dma_start(iden[:], t_iden[:])
            lp = cpool.tile([1, DEPTH * 4], f32)
            nc.sync.dma_start(lp[:], t_lp[:])
            caff = cpool.tile([OC, 4, 2], f32)
            nc.sync.dma_start(caff[:], t_caff[:])
            pos = cpool.tile([128, NCH], f32)
            nc.sync.dma_start(pos[:], t_pos[:])

            # =================== CNN frontend ===================
            # patch embed -> hext0 tiles [128, 10, B]
            xT = cpool.tile([P_IN, W_IN * B], f32)
            nc.sync.dma_start(xT[:], t_xT[:])
            pwT = cpool.tile([P_IN, D], f32)
            nc.sync.dma_start(pwT[:], t_pwT[:])
            pb = cpool.tile([128, NCH], f32)
            nc.sync.dma_start(pb[:], t_pb[:])

            hext = []
            for j in range(NCH):
                hx = npool.tile([128, 10, B], f32, name=f"hx0_{j}", tag=f"hx{j}")
                nc.sync.dma_start(hx[:, 0:4, :], t_cst[0, j])
                pe_ps = ps2pool.tile([128, W_IN, B], f32, name=f"pe_{j}", tag="pe")
                nc.tensor.matmul(pe_ps[:], pwT[:, j * 128:(j + 1) * 128], xT[:],
                                 start=True, stop=True)
                nc.scalar.activation(hx[:, 4:10, :], pe_ps[:], AF.Identity,
                                     bias=pb[:, j:j + 1], scale=1.0)
                hext.append(hx)

            def cbr(layer, hin, L, pool_k):
                """conv(+bn+relu) layer on hext tiles hin (each [128, 4+L, B]).
                Returns sharded output [OC, Lout, B] tile (after optional pool)."""
                cw = npool.tile([128, K, NCH, OC], f32, name=f"cw_{layer}", tag="cw")
                nc.sync.dma_start(cw[:], t_cwT[layer])
                cps = ps2pool.tile([OC, L, B], f32, name=f"cps_{layer}", tag="cps")
                nmm = K * NCH
                m = 0
                for t in range(K):
                    for j in range(NCH):
                        nc.tensor.matmul(cps[:],
                                         cw[:, t, j, :],
                                         hin[j][:, t:t + L, :],
                                         start=(m == 0), stop=(m == nmm - 1))
                        m += 1
                act = npool.tile([OC, L, B], f32, name=f"cbr_{layer}", tag="cbr")
                nc.scalar.activation(act[:], cps[:], AF.Relu,
                                     bias=caff[:, layer, 1:2],
                                     scale=caff[:, layer, 0:1])
                if pool_k == 3:
                    v = act[:].rearrange("p (g k) b -> p g k b", k=3)
                    pooled = npool.tile([OC, L // 3, B], f32,
                                        name=f"pool_{layer}", tag="pool")
                    nc.vector.tensor_max(pooled[:], v[:, :, 0, :], v[:, :, 1, :])
                    nc.vector.tensor_max(pooled[:], pooled[:], v[:, :, 2, :])
                    return pooled, L // 3
                if pool_k == 2:
                    pooled = npool.tile([OC, L // 2, B], f32,
                                        name=f"pool_{layer}", tag="pool")
                    nc.vector.tensor_max(pooled[:], act[:, 0:1, :], act[:, 1:2, :])
                    return pooled, L // 2
                return act, L

            def cnn_allgather(layer, shard, Lout, next_layer):
                """AllGather sharded [OC, Lout, B] -> full; returns new hext tiles
                [128, 4+Lout, B] with next conv layer's state, or the raw gathered
                DRAM tile when next_layer is None."""
                agi = dpool.tile([OC, Lout * B], f32, name=f"agi_{layer}")
                nc.sync.dma_start(agi[:], shard[:].rearrange("p l b -> p (l b)"))
                ago = dpool.tile([D, Lout * B], f32, name=f"ago_{layer}")
                nc.gpsimd.collective_compute(
                    "AllGather", ALU.bypass, replica_groups=RG,
                    ins=[agi.opt()], outs=[ago.opt()])
                if next_layer is None:
                    return ago
                tiles = []
                for j in range(NCH):
                    hx = npool.tile([128, 4 + Lout, B], f32,
                                    name=f"hx{next_layer}_{j}", tag=f"hx{j}")
                    nc.sync.dma_start(hx[:, 0:4, :], t_cst[next_layer, j])
                    nc.sync.dma_start(hx[:, 4:4 + Lout, :],
                                      ago[j * 128:(j + 1) * 128, :])
                    tiles.append(hx)
                return tiles

            s0, L = cbr(0, hext, 6, 0)
            hext = cnn_allgather(0, s0, L, 1)
            s1, L = cbr(1, hext, 6, 3)           # maxpool3 -> L=2
            hext = cnn_allgather(1, s1, L, 2)
            s2, L = cbr(2, hext, 2, 0)
            hext = cnn_allgather(2, s2, L, 3)
            s3, L = cbr(3, hext, 2, 2)           # maxpool2 -> L=1
            ago = cnn_allgather(3, s3, L, None)  # [768, 32] in DRAM

            # tok = gathered + pos_embed row  (channel-major [128, 6, B])
            tok = apool.tile([128, NCH, B], f32)
            tok_raw = apool.tile([128, NCH, B], f32)
            nc.sync.dma_start(tok_raw[:],
                              ago[:].rearrange("(j p) b -> p j b", p=128))
            for j in range(NCH):
                nc.scalar.activation(tok[:, j, :], tok_raw[:, j, :], AF.Identity,
                                     bias=pos[:, j:j + 1], scale=1.0)

            # =================== Mamba layers ===================
            for i in range(DEPTH):
                w1 = wpool.tile([128, NCH, 768], f32, name=f"w1_{i}", tag="w1")
                nc.sync.dma_start(w1[:], t_w1T[i].rearrange("j p c -> p j c"))
                w2a = wpool.tile([128, 768], f32, name=f"w2a_{i}", tag="w2a")
                nc.sync.dma_start(w2a[:], t_w2T[i, 0:128])
                w2b = wpool.tile([64, 768], f32, name=f"w2b_{i}", tag="w2b")
                nc.sync.dma_start(w2b[:], t_w2T[i, 128:192])
                ssm = wpool.tile([DS, B, HD], f32, name=f"ssm_{i}", tag="ssm")
                nc.sync.dma_start(ssm[:], t_ssmT[i].rearrange("b n p -> n b p"))
                cp = wpool.tile([128, 4, K], f32, name=f"cp_{i}", tag="cp")
                nc.sync.dma_start(cp[:], t_cp[i])
                mcs = []
                for g in range(4):
                    mt = wpool.tile([128, 3, B], f32, name=f"mcs_{i}_{g}",
                                    tag=f"mcs{g}")
                    nc.sync.dma_start(mt[:], t_mcs[i, g])
                    mcs.append(mt)

                # --- in_proj: zx psum [128, 6, B]
                # col layout: c0 z0:128 | c1 z128:192+pad | c2 x0:128 |
                #             c3 x128:192+pad+dt@127 | c4 B | c5 C
                zx = pspool.tile([128, NCH, B], f32, name=f"zx_{i}", tag="zx")
                for j in range(NCH):
                    for oc in range(6):
                        nc.tensor.matmul(zx[:, oc, :],
                                         w1[:, j, oc * 128:(oc + 1) * 128],
                                         tok[:, j, :],
                                         start=(j == 0 and oc == 0),
                                         stop=(j == NCH - 1 and oc == 5),
                                         skip_group_check=True)

                # --- z silu
                zsA = apool.tile([128, B], f32, name=f"zsA_{i}", tag="zsA")
                nc.scalar.activation(zsA[:], zx[:, 0, :], AF.Silu, bias=zero_c[:])
                zsB = apool.tile([64, B], f32, name=f"zsB_{i}", tag="zsB")
                nc.scalar.activation(zsB[:], zx[0:64, 1, :], AF.Silu, bias=zero_c[0:64, :])

                # --- dt -> dt_s = ln(1+exp(dt+dtb)), dA = exp(-exp(alog)*dt_s)
                dtr = apool.tile([1, B], f32, name=f"dtr_{i}", tag="dtr")
                nc.scalar.activation(dtr[:], zx[96:97, 3, :], AF.Copy)
                e_t = apool.tile([1, B], f32, name=f"et_{i}", tag="et")
                nc.scalar.activation(e_t[:], dtr[:], AF.Exp,
                                     bias=lp[:, 4 * i:4 * i + 1], scale=1.0)
                dt_s = apool.tile([1, B], f32, name=f"dts_{i}", tag="dts")
                nc.scalar.activation(dt_s[:], e_t[:], AF.Ln,
                                     bias=ones_c[0:1, :], scale=1.0)
                dA = apool.tile([1, B], f32, name=f"dA_{i}", tag="dA")
                nc.scalar.activation(dA[:], dt_s[:], AF.Exp, bias=zero_c[0:1, :],
                                     scale=lp[:, 4 * i + 1:4 * i + 2])

                # --- conv over 4 taps (3 state + new), per group
                def conv_group(g, src_ap, width, name):
                    st = mcs[g]
                    acc = apool.tile([width, B], f32, name=f"cacc{name}_{i}",
                                     tag=f"cacc{g}")
                    nc.vector.tensor_scalar(acc[:], st[0:width, 0, :],
                                            cp[0:width, g, 0:1], None, ALU.mult)
                    for tp in (1, 2):
                        nc.vector.scalar_tensor_tensor(
                            acc[:], st[0:width, tp, :], cp[0:width, g, tp:tp + 1],
                            acc[:], ALU.mult, ALU.add)
                    nc.vector.scalar_tensor_tensor(
                        acc[:], src_ap, cp[0:width, g, 3:4],
                        acc[:], ALU.mult, ALU.add)
                    out = apool.tile([width, B], f32, name=f"cv{name}_{i}",
                                     tag=f"cv{g}")
                    nc.scalar.activation(out[:], acc[:], AF.Silu,
                                         bias=cp[0:width, g, 4:5], scale=1.0)
                    return out

                xhA = conv_group(0, zx[:, 2, :], 128, "xA")
                xhB = conv_group(1, zx[0:64, 3, :], 64, "xB")
                Bm = conv_group(2, zx[:, 4, :], 128, "B")
                Cm = conv_group(3, zx[:, 5, :], 128, "C")

                msc = pspool.tile([128, 5, B], f32, name=f"msc_{i}", tag="msc")
                # dA broadcast to all partitions: ones_col(128) x dA row
                nc.tensor.matmul(msc[:, 1, :], ones_r[:], dA[:],
                                 start=True, stop=False, skip_group_check=True)
                CmP = apool.tile([DS, B], f32, name=f"CmP_{i}", tag="CmP")
                nc.vector.tensor_mul(CmP[:], Cm[:], msc[:, 1, :])

                # --- SSM matvec: y0[p, b] = sum_n ssm[n, b, p] * CmP[n, b]
                y0 = pspool.tile([128, 2, B], f32, name=f"y0_{i}", tag="y0")
                for b in range(B):
                    nc.tensor.matmul(y0[:, 0, b:b + 1], ssm[:, b, 0:128],
                                     CmP[:, b:b + 1], start=(b == 0),
                                     stop=False, skip_group_check=True)
                    nc.tensor.matmul(y0[0:64, 1, b:b + 1], ssm[:, b, 128:192],
                                     CmP[:, b:b + 1], start=False,
                                     stop=(b == B - 1), skip_group_check=True)

                # --- c2 = dt_s * (Bm . Cm) + dpar
                pbc = apool.tile([DS, B], f32, name=f"pbc_{i}", tag="pbc")
                nc.vector.tensor_mul(pbc[:], Bm[:], Cm[:])
                nc.tensor.matmul(msc[0:1, 0, :], ones_c[:], pbc[:],
                                 start=False, stop=False, skip_group_check=True)
                c2 = apool.tile([1, B], f32, name=f"c2_{i}", tag="c2")
                nc.vector.tensor_mul(c2[:], dt_s[:], msc[0:1, 0, :])
                nc.vector.tensor_scalar(c2[:], c2[:],
                                        lp[:, 4 * i + 2:4 * i + 3], None, ALU.add)
                nc.tensor.matmul(msc[:, 2, :], ones_r[:], c2[:],
                                 start=False, stop=False, skip_group_check=True)

                # --- y = (y0 + c2*xh) * silu(z);  ssq = sum_c y^2
                yA = apool.tile([128, B], f32, name=f"yA_{i}", tag="yA")
                nc.vector.tensor_mul(yA[:], xhA[:], msc[:, 2, :])
                nc.vector.tensor_add(yA[:], yA[:], y0[:, 0, :])
                nc.vector.tensor_mul(yA[:], yA[:], zsA[:])
                yB = apool.tile([64, B], f32, name=f"yB_{i}", tag="yB")
                nc.vector.tensor_mul(yB[:], xhB[:], msc[0:64, 2, :])
                nc.vector.tensor_add(yB[:], yB[:], y0[0:64, 1, :])
                nc.vector.tensor_mul(yB[:], yB[:], zsB[:])

                sqA = apool.tile([128, B], f32, name=f"sqA_{i}", tag="sqA")
                nc.vector.tensor_mul(sqA[:], yA[:], yA[:])
                sqB = apool.tile([64, B], f32, name=f"sqB_{i}", tag="sqB")
                nc.vector.tensor_mul(sqB[:], yB[:], yB[:])
                nc.tensor.matmul(msc[0:1, 3, :], ones_c[:], sqA[:],
                                 start=False, stop=False, skip_group_check=True)
                nc.tensor.matmul(msc[0:1, 3, :], ones_c[0:64, :], sqB[:],
                                 start=False, stop=False, skip_group_check=True)

                # --- out_proj partials: u[o, b] (768 rows, 6 chunks)
                u = pspool.tile([128, NCH, B], f32, name=f"u_{i}", tag="u")
                for oc in range(6):
                    nc.tensor.matmul(u[:, oc, :], w2a[:, oc * 128:(oc + 1) * 128],
                                     yA[:], start=(oc == 0), stop=False,
                                     skip_group_check=True)
                    nc.tensor.matmul(u[:, oc, :], w2b[:, oc * 128:(oc + 1) * 128],
                                     yB[:], start=False, stop=(oc == 5),
                                     skip_group_check=True)

                # --- AllReduce [769, B]: rows 0:768 = u (j-major), row 768 = ssq
                ar_sb = apool.tile([128, NCH, B], f32, name=f"arsb_{i}", tag="arsb")
                nc.vector.tensor_copy(ar_sb[:], u[:])
                ssq_sb = apool.tile([1, B], f32, name=f"ssqsb_{i}", tag="ssqsb")
                nc.vector.tensor_copy(ssq_sb[:], msc[0:1, 3, :])
                ari = dpool.tile([D + 1, B], f32, name=f"ari_{i}")
                nc.sync.dma_start(
                    ari[0:D, :].rearrange("(j p) b -> p j b", p=128), ar_sb[:])
                nc.sync.dma_start(ari[D:D + 1, :], ssq_sb[:])
                aro = dpool.tile([D + 1, B], f32, name=f"aro_{i}")
                nc.gpsimd.collective_compute(
                    "AllReduce", ALU.add, replica_groups=RG,
                    ins=[ari.opt()], outs=[aro.opt()])
                ut = apool.tile([128, NCH, B], f32, name=f"ut_{i}", tag="ut")
                nc.sync.dma_start(ut[:],
                                  aro[0:D, :].rearrange("(j p) b -> p j b", p=128))
                qt = apool.tile([1, B], f32, name=f"qt_{i}", tag="qt")
                nc.sync.dma_start(qt[:], aro[D:D + 1, :])

                # --- r = 1/sqrt(ssq/1536 + eps); tok += r * u
                vv = apool.tile([1, B], f32, name=f"vv_{i}", tag="vv")
                nc.scalar.activation(vv[:], qt[:], AF.Ln,
                                     bias=eps_c[0:1, :], scale=1.0 / DI)
                rr = apool.tile([1, B], f32, name=f"rr_{i}", tag="rr")
                nc.scalar.activation(rr[:], vv[:], AF.Exp,
                                     bias=zero_c[0:1, :], scale=-0.5)
                nc.tensor.matmul(msc[:, 4, :], ones_r[:], rr[:],
                                 start=False, stop=True, skip_group_check=True)
                for j in range(NCH):
                    tmp = apool.tile([128, B], f32, name=f"tu_{i}_{j}", tag="tu")
                    nc.vector.tensor_mul(tmp[:], ut[:, j, :], msc[:, 4, :])
                    nc.vector.tensor_add(tok[:, j, :], tok[:, j, :], tmp[:])

            # =================== head ===================
            fcn = cpool.tile([128, NCH, 2], f32)
            nc.sync.dma_start(fcn[:], t_fcn[:])
            hist = cpool.tile([128, NCH, B], f32)
            nc.sync.dma_start(hist[:], t_hist[:])
            fcwT = cpool.tile([128, NCH, NCLS], f32)
            nc.sync.dma_start(fcwT[:], t_fcwT[:])
            fcb = cpool.tile([1, NCLS], f32)
            nc.sync.dma_start(fcb[:], t_fcb[:])

            hm = pspool.tile([128, 5, B], f32, name="hm", tag="msc")
            for j in range(NCH):
                nc.tensor.matmul(hm[:, 0, :], ones_f[:], tok[:, j, :],
                                 start=(j == 0), stop=False,
                                 skip_group_check=True)
            sq6 = apool.tile([128, NCH, B], f32, name="sq6")
            nc.vector.tensor_mul(sq6[:], tok[:], tok[:])
            for j in range(NCH):
                nc.tensor.matmul(hm[:, 1, :], ones_f[:], sq6[:, j, :],
                                 start=False, stop=False, skip_group_check=True)
            mu = apool.tile([1, B], f32, name="mu")
            nc.vector.tensor_scalar(mu[:], hm[0:1, 0, :], 1.0 / D, None, ALU.mult)
            mu2 = apool.tile([1, B], f32, name="mu2")
            nc.vector.tensor_mul(mu2[:], mu[:], mu[:])
            var = apool.tile([1, B], f32, name="var")
            nc.vector.tensor_scalar(var[:], hm[0:1, 1, :], 1.0 / D, None, ALU.mult)
            nc.vector.tensor_sub(var[:], var[:], mu2[:])
            sig = apool.tile([1, B], f32, name="sig")
            nc.scalar.activation(sig[:], var[:], AF.Ln, bias=eps_c[0:1, :], scale=1.0)
            isig = apool.tile([1, B], f32, name="isig")
            nc.scalar.activation(isig[:], sig[:], AF.Exp,
                                 bias=zero_c[0:1, :], scale=-0.5)
            nc.tensor.matmul(hm[:, 2, :], ones_r[:], isig[:], start=False,
                             stop=True, skip_group_check=True)

            hn = apool.tile([128, NCH, B], f32, name="hn")
            for j in range(NCH):
                tn = apool.tile([128, B], f32, name=f"tn_{j}", tag="tn")
                nc.vector.scalar_tensor_tensor(tn[:], hm[:, 0, :], -1.0 / D,
                                               tok[:, j, :], ALU.mult, ALU.add)
                nc.vector.tensor_mul(tn[:], tn[:], hm[:, 2, :])
                # t_norm = tn * g + b
                nc.scalar.activation(tn[:], tn[:], AF.Identity,
                                     bias=fcn[:, j, 1:2], scale=fcn[:, j, 0:1])
                # history = alpha*hist + beta*t_norm  (hist pre-scaled by alpha,
                # beta baked on host into lp slot? -> use immediate via host const)
                nc.vector.tensor_add(hn[:, j, :], tn[:], hist[:, j, :])

            lg = ps2pool.tile([B, NCLS], f32, name="lg", tag="lg")
            for j in range(NCH):
                nc.tensor.matmul(lg[:], hn[:, j, :], fcwT[:, j, :],
                                 start=(j == 0), stop=False)
            nc.tensor.matmul(lg[:], ones_r[:, 0:B], fcb[:], start=False, stop=True)
            lg_sb = apool.tile([B, NCLS], f32, name="lg_sb")
            nc.vector.tensor_copy(lg_sb[:], lg[:])
            nc.sync.dma_start(t_logits[:], lg_sb[:])

            ho_sb = apool.tile([B, NCH, 128], f32, name="ho_sb")
            for j in range(NCH):
                hp = ps2pool.tile([B, 128], f32, name=f"hp_{j}", tag="hp")
                nc.tensor.transpose(hp[:], hn[:, j, :], iden[:])
                nc.vector.tensor_copy(ho_sb[:, j, :], hp[:])
            nc.sync.dma_start(t_histo[:], ho_sb[:].rearrange("p j c -> p (j c)"))

    nc.compile()
    _built['nc'] = nc
    return nc


def _prep_inputs(inputs):
    """Host-side: shard, transpose, fold. Returns in_maps (one dict/core)."""
    f = np.float32
    x = np.asarray(inputs['x'], f)                    # [B,1,16,6]
    pos_idx = int(inputs['position_index'])
    clip = min(pos_idx, 299)

    xT = np.ascontiguousarray(x[:, 0].transpose(1, 2, 0)).reshape(P_IN, W_IN * B)
    pwT = np.ascontiguousarray(np.asarray(inputs['patch_w'], f).T)     # [16,768]
    pb = np.ascontiguousarray(np.asarray(inputs['patch_b'], f).reshape(NCH, 128).T)

    cnn_w = np.asarray(inputs['cnn_w'], f)            # [4,768,768,5]
    bn_g = np.asarray(inputs['bn_g'], f); bn_b = np.asarray(inputs['bn_b'], f)
    bn_rm = np.asarray(inputs['bn_rm'], f); bn_rv = np.asarray(inputs['bn_rv'], f)
    cnn_b = np.asarray(inputs['cnn_b'], f)
    s = bn_g / np.sqrt(bn_rv + 1e-5)                  # [4,768]
    caff_full = np.stack([s, (cnn_b - bn_rm) * s + bn_b], axis=-1)  # [4,768,2]

    cst_full = np.asarray(inputs['cnn_state'], f)     # [4,B,768,4]
    # -> [4, NCH, 128, 4, B]
    cst = np.ascontiguousarray(
        cst_full.transpose(0, 2, 3, 1).reshape(4, NCH, 128, 4, B))

    pos_row = np.asarray(inputs['pos_embed'], f)[0, 1 + clip]  # [768]
    pos = np.ascontiguousarray(pos_row.reshape(NCH, 128).T)

    in_proj = np.asarray(inputs['in_proj_w'], f)      # [12, 3336, 768]
    conv_w = np.asarray(inputs['conv1d_w'], f)        # [12, 1792, 4]
    conv_b = np.asarray(inputs['conv1d_b'], f)        # [12, 1792]
    A_log = np.asarray(inputs['A_log'], f)
    dt_bias = np.asarray(inputs['dt_bias'], f)
    D_param = np.asarray(inputs['D_param'], f)
    norm_w = np.asarray(inputs['norm_w'], f)          # [12, 1536]
    out_proj = np.asarray(inputs['out_proj_w'], f)    # [12, 768, 1536]
    mcs_full = np.asarray(inputs['mamba_conv_state'], f)  # [12,B,1792,4]
    ssm_full = np.asarray(inputs['ssm_state'], f)     # [12,B,8,192,128]

    hist_f = np.asarray(inputs['history_feature'], f)[:, 0]  # [B,768]
    alpha = pos_idx / (pos_idx + 1.0)
    histT = np.ascontiguousarray((hist_f * alpha).T.reshape(NCH, 128, B)
                                 .transpose(1, 0, 2))  # [128, NCH, B]
    fcn_g = np.asarray(inputs['fcn_g'], f); fcn_b = np.asarray(inputs['fcn_b'], f)
    beta = 1.0 / (pos_idx + 1.0)
    fcn = np.ascontiguousarray(
        np.stack([(fcn_g * beta).reshape(NCH, 128).T,
                  (fcn_b * beta).reshape(NCH, 128).T], axis=-1))
    fc_w = np.asarray(inputs['fc_w'], f)              # [100, 768]
    fcwT = np.ascontiguousarray(
        fc_w.T.reshape(NCH, 128, NCLS).transpose(1, 0, 2))  # [128, NCH, 100]
    fcb = np.asarray(inputs['fc_b'], f).reshape(1, NCLS)
    iden = np.eye(128, dtype=f)

    in_maps = []
    for h in range(NCORES):
        m = {}
        m['xT'] = xT; m['pwT'] = pwT; m['pb'] = pb
        # cnn weights: core h owns out-channels [OC*h, OC*(h+1))
        osl = slice(OC * h, OC * (h + 1))
        # cwT[l, t, j, p, o] = cnn_w[l, OC*h+o, 128j+p, t]
        cw = cnn_w[:, osl]                              # [4, 96, 768, 5]
        m['cwT'] = np.ascontiguousarray(
            cw.reshape(4, OC, NCH, 128, K).transpose(0, 3, 4, 2, 1))
        m['caff'] = np.ascontiguousarray(caff_full[:, osl].transpose(1, 0, 2))
        m['cst'] = cst
        m['pos'] = pos

        # in_proj col layout per core:
        # [z 192 | pad 64 | x 192 | pad 63 | dt 1 | B 128 | C 128] = 768 cols
        w1 = np.zeros((DEPTH, 768, 768), f)            # [l, k, col] (k=contract)
        zsl = slice(HD * h, HD * (h + 1))
        w1[:, :, 0:192] = in_proj[:, zsl].transpose(0, 2, 1)
        xsl = slice(DI + HD * h, DI + HD * (h + 1))
        w1[:, :, 256:448] = in_proj[:, xsl].transpose(0, 2, 1)
        w1[:, :, 480] = in_proj[:, 2 * DI + 2 * DS + h]         # dt row
        w1[:, :, 512:640] = in_proj[:, DI + DI:DI + DI + DS].transpose(0, 2, 1)
        w1[:, :, 640:768] = in_proj[:, DI + DI + DS:DI + DI + 2 * DS] \
            .transpose(0, 2, 1)
        m['w1T'] = np.ascontiguousarray(
            w1.reshape(DEPTH, NCH, 128, 768))

        # conv params: groups [x0:128, x128:192(+pad), B, C] rows x [cw0..3, cb]
        cpk = np.zeros((DEPTH, 128, 4, K), f)
        ch_x = slice(HD * h, HD * h + HD)
        cw_x = conv_w[:, ch_x]; cb_x = conv_b[:, ch_x]
        cpk[:, :, 0, 0:4] = cw_x[:, 0:128]; cpk[:, :, 0, 4] = cb_x[:, 0:128]
        cpk[:, 0:64, 1, 0:4] = cw_x[:, 128:192]; cpk[:, 0:64, 1, 4] = cb_x[:, 128:192]
        cpk[:, :, 2, 0:4] = conv_w[:, DI:DI + DS]
        cpk[:, :, 2, 4] = conv_b[:, DI:DI + DS]
        cpk[:, :, 3, 0:4] = conv_w[:, DI + DS:DI + 2 * DS]
        cpk[:, :, 3, 4] = conv_b[:, DI + DS:DI + 2 * DS]
        m['cp'] = np.ascontiguousarray(cpk)

        # mamba conv state taps 1..3: [DEPTH, 4, 128, 3, B]
        mk = np.zeros((DEPTH, 4, 128, 3, B), f)
        st_x = mcs_full[:, :, ch_x, 1:4]               # [12, B, 192, 3]
        mk[:, 0, :, :, :] = st_x[:, :, 0:128].transpose(0, 2, 3, 1)
        mk[:, 1, 0:64] = st_x[:, :, 128:192].transpose(0, 2, 3, 1)
        mk[:, 2] = mcs_full[:, :, DI:DI + DS, 1:4].transpose(0, 2, 3, 1)
        mk[:, 3] = mcs_full[:, :, DI + DS:DI + 2 * DS, 1:4].transpose(0, 2, 3, 1)
        m['mcs'] = np.ascontiguousarray(mk)

        # ssmT[l, b, n, p] = ssm[l, b, h, p, n]
        m['ssmT'] = np.ascontiguousarray(
            ssm_full[:, :, h].transpose(0, 1, 3, 2))

        # w2T[l, c, o] = out_proj[l, o, 192h + c] * norm_w[l, 192h + c]
        w2 = out_proj[:, :, zsl] * norm_w[:, None, zsl]
        m['w2T'] = np.ascontiguousarray(w2.transpose(0, 2, 1))

        lpk = np.zeros((1, DEPTH * 4), f)
        lpk[0, 0::4] = dt_bias[:, h]
        lpk[0, 1::4] = -np.exp(A_log[:, h])
        lpk[0, 2::4] = D_param[:, h]
        m['lp'] = lpk

        m['fcn'] = fcn; m['histT'] = histT; m['fcwT'] = fcwT; m['fcb'] = fcb
        m['iden'] = iden
        in_maps.append(m)
    return in_maps


def kernel(**inputs):
    from concourse.bass_utils import run_bass_kernel_spmd
    nc = _build()
    in_maps = _prep_inputs(inputs)
    res = run_bass_kernel_spmd(nc, in_maps, list(range(NCORES)))
    out = res.results[0]
    logits = np.asarray(out['logits']).reshape(B, 1, NCLS).astype(np.float32)
    history = np.asarray(out['history']).reshape(B, 1, D).astype(np.float32)
    return logits, history


def profile(tmpdir="/tmp/ntff_kernel", **inputs):
    """Run with NTFF tracing; returns BassKernelResults (exec_time_ns set)."""
    from concourse.bass_utils import run_bass_kernel_spmd
    nc = _build()
    in_maps = _prep_inputs(inputs)
    return run_bass_kernel_spmd(nc, in_maps, list(range(NCORES)),
                                trace=True, tmpdir=tmpdir)
